# revision 30
# baseline (speedup 1.0000x reference)
"""3-layer GCN encoder (nn_GCNEncoder) on 8 Trainium2 NeuronCores.

Strategy (graph/data parallel, 1D node sharding), v2:
  - Node shard c = rows [c*NPC, (c+1)*NPC).  Core c owns all edges whose
    *destination* lies in its shard (plus that shard's self-loops).
  - GCN norm is factorized:  out = dinv ⊙ (A^T (dinv ⊙ (h W))) + b, so no
    per-edge scaling is needed: dinv is applied once per node before the
    AllGather (on the transform output) and once per node at PSUM
    evacuation.
  - Per layer on each core:
      1. transform:  u = dinv ⊙ (h @ W)
      2. AllGather u  ->  u_full [N, F] in local HBM (the gather table)
      3. aggregation: edges sorted by (superblock, src-group, dst-block,
         dst) and PACKED per (superblock, group) run: chunks of 128 edge
         slots may span dst-block boundaries.  dma_gather pulls the source
         rows (src-group slicing keeps gather indices within int16); each
         128-slot chunk is scattered into per-dst-block PSUM accumulators
         via one matmul per (chunk, block) "part":  psum[F, dst] +=
         msg^T @ S_part, where S_part is a 0/1 selection matrix built on
         DVE (is_equal against an iota constant) whose rows are zero for
         slots not belonging to that block.  Packing the run (instead of
         padding each (block, group) bucket to a chunk multiple) pools the
         max-over-cores variance and cuts gathered slots by ~13%.
      4. evacuation: h_next = relu(dinv ⊙ psum + b)
  - The SPMD instruction stream is shared by all 8 cores: run slot counts
    are the max over cores; pad slots gather row 0 with a -1 label (zero
    row in every S), contributing nothing.

kernel() takes the full unsharded inputs and returns the full output.
"""

import os
import sys

import numpy as np

sys.path.insert(0, "/opt/trn_rl_repo")

P = 128
GMAX = 1024          # max idxs per dma_gather (>=2048 wedges the NC)


class Cfg:
    def __init__(self, n_nodes, n_cores, d_in, d_hid, d_out,
                 sb_blocks=6, n_groups=4):
        assert n_nodes % n_cores == 0
        self.n_nodes = n_nodes
        self.n_cores = n_cores
        self.d_in, self.d_hid, self.d_out = d_in, d_hid, d_out
        self.npc = n_nodes // n_cores              # nodes per core
        self.nblk = -(-self.npc // P)              # dst blocks per core
        self.npcp = self.nblk * P                  # padded nodes per core
        self.sb_blocks = sb_blocks                 # dst blocks per superblock
        self.nsb = -(-self.nblk // sb_blocks)
        self.n_groups = n_groups                   # src-range groups
        assert n_cores % n_groups == 0
        self.cpg = n_cores // n_groups             # cores per src group
        self.grp = self.npcp * self.cpg            # padded rows per src group
        assert self.grp <= 32767, "src group must fit int16 gather indices"


def _host_prep(edge_index, cfg):
    """Shard edges, build the shared packed-run schedule and per-core data.

    Returns (sched, per_core).  sched (shared across cores):
      - runs: list per (sb, g) of dict(slots, gathers=[(n_idxs, n_chunks)],
              parts=[(chunk_in_run, block, stop)], icol0, pcol0)
      - tot16: total idx plane columns;  totparts: total part count
      - dinv
    per_core[c]: idx plane [128, tot16] int16, dstl plane [128, totparts] f32
    """
    n, ncores, npc = cfg.n_nodes, cfg.n_cores, cfg.npc
    ei = np.asarray(edge_index)
    src = ei[0]
    dst = ei[1]
    deg = (np.bincount(dst, minlength=n) + 1).astype(np.float64)
    dinv = (1.0 / np.sqrt(deg)).astype(np.float32)

    core = dst // npc
    # per-core sorted edge arrays and per-(sb,g,b) counts
    per_core_raw = []
    counts = np.zeros((ncores, cfg.nblk, cfg.n_groups), dtype=np.int64)
    for c in range(ncores):
        m = core == c
        s = src[m]
        d = (dst[m] - c * npc).astype(np.int64)
        blk = d // P
        sc = s // npc
        grp = sc // cfg.cpg
        sb = blk // cfg.sb_blocks
        order = np.lexsort((d, blk, grp, sb))
        s, d, blk, grp = s[order], d[order], blk[order], grp[order]
        sc = s // npc
        loc = (sc % cfg.cpg) * cfg.npcp + (s % npc)
        np.add.at(counts[c], (blk, grp), 1)
        per_core_raw.append((loc, d, blk, grp))

    # shared run schedule: per (sb, g) slots = 128*ceil(max_c count/128)
    runs = []
    tot16 = 0
    totparts = 0
    last_part_of_block = {}   # (block) -> part index (global)
    all_parts = []            # global part list in stream order
    for sb in range(cfg.nsb):
        blocks = list(range(sb * cfg.sb_blocks,
                            min((sb + 1) * cfg.sb_blocks, cfg.nblk)))
        for g in range(cfg.n_groups):
            cnt_cb = counts[:, blocks, g]            # [ncores, nb]
            run_max = int(cnt_cb.sum(axis=1).max())
            nch = max(1, -(-run_max // P))
            slots = nch * P
            # union over cores of (chunk, block) parts
            parts_set = set()
            for c in range(ncores):
                cum = 0
                for bi, b in enumerate(blocks):
                    cnt = int(counts[c, b, g])
                    if cnt == 0:
                        cum += 0
                        continue
                    lo, hi = cum, cum + cnt
                    for k in range(lo // P, -(-hi // P)):
                        parts_set.add((k, b))
                    cum = hi
            parts = sorted(parts_set)
            part_list = []
            for (k, b) in parts:
                gidx = len(all_parts)
                part_list.append([k, b, False])
                all_parts.append([sb, g, k, b])
                last_part_of_block[b] = gidx
            # gathers: split slots into <=GMAX idx pieces (multiples of 128)
            gathers = []
            rem = slots
            while rem > 0:
                take = min(GMAX, rem)
                gathers.append((take, take // P))
                rem -= take
            runs.append({
                "sb": sb, "g": g, "blocks": blocks, "slots": slots,
                "nch": nch, "gathers": gathers,
                "parts": part_list, "icol0": tot16, "pcol0": totparts,
            })
            tot16 += slots // 16
            totparts += len(part_list)
    # stop flags: statically-last part of each block
    for b, gidx in last_part_of_block.items():
        sb, g, k, bb = all_parts[gidx]
        for run in runs:
            if run["sb"] == sb and run["g"] == g:
                for pl in run["parts"]:
                    if pl[0] == k and pl[1] == b:
                        pl[2] = True

    # per-core planes
    per_core = []
    for c in range(ncores):
        loc, d, blk, grp = per_core_raw[c]
        # order index into sorted arrays by (sb, g): recompute segment ptrs
        sbv = blk // cfg.sb_blocks
        key = sbv * cfg.n_groups + grp
        # edges already sorted by (sb, g, blk, d) => key is nondecreasing
        idx_plane = np.zeros((16, tot16), dtype=np.int16)
        dstl_plane = np.full((P, totparts), -1.0, dtype=np.float32)
        ptr = 0
        for run in runs:
            sb, g = run["sb"], run["g"]
            kk = sb * cfg.n_groups + g
            lo = ptr + np.searchsorted(key[ptr:], kk, side="left")
            hi = ptr + np.searchsorted(key[ptr:], kk, side="right")
            ptr = hi
            cnt = hi - lo
            slots = run["slots"]
            assert cnt <= slots
            arr = np.zeros(slots, dtype=np.int16)
            arr[:cnt] = loc[lo:hi].astype(np.int16)
            a16 = arr.reshape(slots // 16, 16).T      # [16, slots/16]
            idx_plane[:, run["icol0"]:run["icol0"] + slots // 16] = a16
            # labels per part
            dl = d[lo:hi]
            bl = blk[lo:hi]
            for j, (k, b, _stop) in enumerate(run["parts"]):
                w0, w1 = k * P, (k + 1) * P
                col = np.full(P, -1.0, dtype=np.float32)
                lo2, hi2 = max(w0, 0), min(w1, cnt)
                if lo2 < hi2:
                    seg = slice(lo2, hi2)
                    mask = bl[seg] == b
                    rel = np.nonzero(mask)[0]
                    if rel.size:
                        col[(lo2 - w0) + rel] = (dl[seg][mask] - b * P)
                dstl_plane[:, run["pcol0"] + j] = col
        assert ptr == len(loc)
        idx_wrapped = np.ascontiguousarray(np.tile(idx_plane, (8, 1)))
        per_core.append({"idx": idx_wrapped,
                         "dstl": np.ascontiguousarray(dstl_plane)})

    maxparts_g = 0
    for run in runs:
        # parts per gather window (for st batch sizing)
        ch0 = 0
        for (n_idx, n_ch) in run["gathers"]:
            pc = sum(1 for (k, b, _s) in run["parts"]
                     if ch0 <= k < ch0 + n_ch)
            maxparts_g = max(maxparts_g, pc)
            ch0 += n_ch
    sched = {
        "runs": runs, "tot16": tot16, "totparts": totparts,
        "maxrun16": max(r["slots"] // 16 for r in runs),
        "maxparts_g": maxparts_g,
        "dinv": dinv,
        "nch_by_block": counts.max(axis=0),   # [nblk, n_groups] edge counts
    }
    return sched, per_core


def build_nc(cfg, sched, debug=False):
    from concourse import bacc, mybir

    f32 = mybir.dt.float32
    bf16 = mybir.dt.bfloat16
    i16 = mybir.dt.int16
    Alu = mybir.AluOpType
    Act = mybir.ActivationFunctionType

    npc, nblk = cfg.npc, cfg.nblk
    tot16, totparts = sched["tot16"], sched["totparts"]
    runs = sched["runs"]
    maxrun16 = sched["maxrun16"]
    maxparts_g = sched["maxparts_g"]
    layer_dims = [(cfg.d_in, cfg.d_hid), (cfg.d_hid, cfg.d_hid),
                  (cfg.d_hid, cfg.d_out)]
    # u is bf16 for all layers; layer 3 (fo=64) pads rows to UC=128 columns
    # so gather rows stay 256B (the 64 garbage columns are never read)
    u_dt = [bf16, bf16, bf16]
    UC = P

    nc = bacc.Bacc("TRN2", target_bir_lowering=False, debug=debug,
                   enable_asserts=False, num_devices=cfg.n_cores)

    xT = nc.dram_tensor("xT", [P, cfg.npcp], f32, kind="ExternalInput")
    Wd, Bd = [], []
    for li, (fi, fo) in enumerate(layer_dims):
        Wd.append(nc.dram_tensor(f"W{li + 1}", [fi, fo], f32, kind="ExternalInput"))
        Bd.append(nc.dram_tensor(f"B{li + 1}", [fo, 1], f32, kind="ExternalInput"))
    dinv_col_d = nc.dram_tensor("dinv_col", [P, nblk], f32, kind="ExternalInput")
    dinvb_d = nc.dram_tensor("dinvb", [P, cfg.npcp], f32, kind="ExternalInput")
    iota_d = nc.dram_tensor("iota_t", [P, maxparts_g * P], f32,
                            kind="ExternalInput")
    ident_d = nc.dram_tensor("ident", [P, P], f32, kind="ExternalInput")
    idx_d = nc.dram_tensor("idxs", [P, tot16], i16, kind="ExternalInput")
    dstl_d = nc.dram_tensor("dstl", [P, totparts], f32, kind="ExternalInput")
    outT = nc.dram_tensor("outT", [cfg.d_out, cfg.npcp], f32,
                          kind="ExternalOutput")

    u_own, u_full = [], []
    for li, (fi, fo) in enumerate(layer_dims):
        u_own.append(nc.dram_tensor(f"u_own{li + 1}", [cfg.npcp, UC],
                                    u_dt[li]))
        u_full.append(nc.dram_tensor(f"u_full{li + 1}",
                                     [cfg.n_cores * cfg.npcp, UC], u_dt[li],
                                     addr_space="Shared"))

    from concourse import tile

    rg = [list(range(cfg.n_cores))]
    with tile.TileContext(nc) as tc:
        with (
            tc.tile_pool(name="const", bufs=1) as constp,
            tc.tile_pool(name="hbuf", bufs=1) as hp,
            tc.tile_pool(name="gath", bufs=4) as gp,
            tc.tile_pool(name="gidx", bufs=3) as ip,
            tc.tile_pool(name="sel", bufs=4) as sp,
            tc.tile_pool(name="dinvb", bufs=2) as dbp,
            tc.tile_pool(name="evac", bufs=3) as tp,
            tc.tile_pool(name="ustage", bufs=3) as up,
            tc.tile_pool(name="accp", bufs=cfg.sb_blocks, space="PSUM") as accp,
            tc.tile_pool(name="auxp", bufs=2, space="PSUM") as auxp,
        ):
            from concourse import library_config
            nc.gpsimd.load_library(library_config.mlp)

            # constants
            wt, bt = [], []
            for li, (fi, fo) in enumerate(layer_dims):
                w = constp.tile([fi, fo], f32, tag=f"w{li}")
                nc.sync.dma_start(w[:], Wd[li][:])
                wt.append(w)
                b = constp.tile([fo, 1], f32, tag=f"b{li}")
                nc.sync.dma_start(b[:], Bd[li][:])
                bt.append(b)
            dct = constp.tile([P, nblk], f32, tag="dct")
            nc.sync.dma_start(dct[:], dinv_col_d[:])
            iot = constp.tile([P, maxparts_g * P], f32, tag="iot")
            nc.sync.dma_start(iot[:], iota_d[:])
            idt = constp.tile([P, P], f32, tag="idt")
            nc.sync.dma_start(idt[:], ident_d[:])
            dlt = constp.tile([P, totparts], f32, tag="dlt")
            nc.sync.dma_start(dlt[:], dstl_d[:])
            # bf16 copies for the bf16 aggregation path (cast on device)
            iot_b = constp.tile([P, maxparts_g * P], bf16, tag="iot_b")
            nc.vector.tensor_scalar_add(iot_b[:], iot[:], 0.0)
            idt_b = constp.tile([P, P], bf16, tag="idt_b")
            nc.vector.tensor_scalar_add(idt_b[:], idt[:], 0.0)
            dlt_b = constp.tile([P, totparts], bf16, tag="dlt_b")
            nc.vector.tensor_scalar_add(dlt_b[:], dlt[:], 0.0)

            h = hp.tile([P, cfg.npcp], f32, tag="h")
            nc.sync.dma_start(h[:], xT[:])

            def emit_transform_block(li, b):
                fi, fo = layer_dims[li]
                off = b * P
                pt = auxp.tile([P, P], f32, tag="aux")
                nc.tensor.matmul(pt[:P, :fo], lhsT=h[:fi, off:off + P],
                                 rhs=wt[li][:, :fo], start=True, stop=True)
                ut = up.tile([P, P], u_dt[li], tag=f"u{li}")
                nc.vector.tensor_scalar_mul(ut[:P, :fo], pt[:P, :fo],
                                            dct[:P, b:b + 1])
                nc.sync.dma_start(u_own[li][off:off + P, :fo], ut[:P, :fo])

            # layer 1 transform (later layers' transforms are interleaved
            # into the previous layer's evacuation loop)
            for b in range(nblk):
                emit_transform_block(0, b)

            for li, (fi, fo) in enumerate(layer_dims):
                last_layer = li == len(layer_dims) - 1
                ud = u_dt[li]
                idt_l = idt if ud == f32 else idt_b
                iot_l = iot if ud == f32 else iot_b
                dlt_l = dlt if ud == f32 else dlt_b

                # ---- AllGather the transformed features ----
                nc.gpsimd.collective_compute(
                    "AllGather", mybir.AluOpType.bypass, replica_groups=rg,
                    ins=[u_own[li][:]], outs=[u_full[li][:]],
                )

                # ---- aggregation (packed runs) ----
                for sb in range(cfg.nsb):
                    blocks = list(range(sb * cfg.sb_blocks,
                                        min((sb + 1) * cfg.sb_blocks, nblk)))
                    # self-loop contribution opens each block's accumulation
                    sb_off = blocks[0] * P
                    nfull = len(blocks)
                    ublk = gp.tile([P, cfg.sb_blocks * P], ud, tag=f"ublk{li}",
                                   name=f"ublk{li}_{sb}")
                    nc.sync.dma_start(
                        ublk[:, :nfull * fo].rearrange(
                            "p (c f) -> p c f", f=fo),
                        u_own[li][sb_off:sb_off + nfull * P, :fo].rearrange(
                            "(c p) f -> p c f", p=P))
                    acc = {}
                    has_parts = {b: False for b in blocks}
                    for run in runs:
                        if run["sb"] == sb:
                            for (k, b, _s) in run["parts"]:
                                has_parts[b] = True
                    for b in blocks:
                        ci = b - blocks[0]
                        acc[b] = accp.tile([P, P], f32, tag="acc",
                                           name=f"acc{li}_{b}")
                        nc.tensor.matmul(
                            acc[b][:fo, :],
                            lhsT=ublk[:, ci * fo:ci * fo + fo],
                            rhs=idt_l[:, :],
                            start=True,
                            stop=not has_parts[b],
                        )
                    for g in range(cfg.n_groups):
                        run = runs[sb * cfg.n_groups + g]
                        slots = run["slots"]
                        l16 = slots // 16
                        it = ip.tile([P, maxrun16], i16, tag="it")
                        nc.scalar.dma_start(
                            it[:, :l16],
                            idx_d[:, run["icol0"]:run["icol0"] + l16])
                        ch0 = 0
                        for (n_idx, n_ch) in run["gathers"]:
                            gt = gp.tile([P, (GMAX // P) * UC], ud,
                                         tag=f"gt{li}")
                            nc.gpsimd.dma_gather(
                                out_ap=gt[:, :n_ch * UC].rearrange(
                                    "p (c e) -> p c e", e=UC),
                                in_ap=u_full[li][g * cfg.grp:(g + 1) * cfg.grp, :],
                                idxs_ap=it[:, ch0 * 8:(ch0 + n_ch) * 8],
                                num_idxs=n_idx,
                                num_idxs_reg=n_idx,
                                elem_size=UC,
                                single_packet=False,
                            )
                            gparts = [(j, k, b, stop) for j, (k, b, stop)
                                      in enumerate(run["parts"])
                                      if ch0 <= k < ch0 + n_ch]
                            if gparts:
                                npg = len(gparts)
                                st = sp.tile([P, maxparts_g * P], ud,
                                             tag=f"st{li}")
                                # batched one-hot build: one is_equal over
                                # all parts of this gather
                                pc0 = run["pcol0"] + gparts[0][0]
                                in1 = dlt_l[:, pc0:pc0 + npg].rearrange(
                                    "p (c o) -> p c o", o=1).to_broadcast(
                                        [P, npg, P])
                                nc.vector.tensor_tensor(
                                    out=st[:, :npg * P].rearrange(
                                        "p (c e) -> p c e", e=P),
                                    in0=iot_l[:, :npg * P].rearrange(
                                        "p (c e) -> p c e", e=P),
                                    in1=in1,
                                    op=Alu.is_equal,
                                )
                                for jj, (j, k, b, stop) in enumerate(gparts):
                                    nc.tensor.matmul(
                                        acc[b][:fo, :],
                                        lhsT=gt[:, (k - ch0) * UC:
                                                (k - ch0) * UC + fo],
                                        rhs=st[:, jj * P:(jj + 1) * P],
                                        start=False, stop=stop,
                                    )
                            ch0 += n_ch

                    # ---- evacuate superblock ----
                    sb_off = blocks[0] * P
                    sb_w = (blocks[-1] + 1) * P - sb_off
                    dbt = dbp.tile([P, cfg.sb_blocks * P], f32, tag="dbt")
                    nc.sync.dma_start(dbt[:, :sb_w],
                                      dinvb_d[:, sb_off:sb_off + sb_w])
                    for b in blocks:
                        off = b * P
                        tt = tp.tile([P, P], f32, tag="tt")
                        nc.vector.tensor_tensor(
                            tt[:fo, :P], in0=acc[b][:fo, :P],
                            in1=dbt[:fo, off - sb_off:off - sb_off + P],
                            op=Alu.mult)
                        if not last_layer:
                            nc.scalar.activation(h[:fo, off:off + P],
                                                 tt[:fo, :P], Act.Relu,
                                                 bias=bt[li][:, :1])
                            # next layer's transform for this block, fed by
                            # the h columns just written
                            emit_transform_block(li + 1, b)
                        else:
                            ot = up.tile([P, P], f32, tag="uo")
                            nc.vector.tensor_scalar_add(ot[:fo, :P],
                                                        tt[:fo, :P],
                                                        bt[li][:, :1])
                            nc.sync.dma_start(outT[:, off:off + P],
                                              ot[:fo, :P])

    nc.finalize()
    return nc


def make_in_maps(x, W1, b1, W2, b2, W3, b3, cfg, sched, per_core):
    x = np.ascontiguousarray(np.asarray(x, dtype=np.float32))
    dinv = sched["dinv"]
    npc, nblk = cfg.npc, cfg.nblk
    iota = np.tile(np.arange(P, dtype=np.float32),
                   (P, sched["maxparts_g"]))
    common = {
        "W1": np.ascontiguousarray(np.asarray(W1, np.float32)),
        "W2": np.ascontiguousarray(np.asarray(W2, np.float32)),
        "W3": np.ascontiguousarray(np.asarray(W3, np.float32)),
        "B1": np.asarray(b1, np.float32).reshape(-1, 1).copy(),
        "B2": np.asarray(b2, np.float32).reshape(-1, 1).copy(),
        "B3": np.asarray(b3, np.float32).reshape(-1, 1).copy(),
        "iota_t": np.ascontiguousarray(iota),
        "ident": np.eye(P, dtype=np.float32),
    }
    in_maps = []
    for c in range(cfg.n_cores):
        dv_pad = np.zeros(cfg.npcp, np.float32)
        dv_pad[:npc] = dinv[c * npc:(c + 1) * npc]
        xT = np.zeros((P, cfg.npcp), np.float32)
        xT[:, :npc] = x[c * npc:(c + 1) * npc].T
        m = dict(common)
        m["xT"] = xT
        m["dinv_col"] = np.ascontiguousarray(dv_pad.reshape(nblk, P).T)
        m["dinvb"] = np.ascontiguousarray(np.broadcast_to(dv_pad, (P, cfg.npcp)))
        m["idxs"] = per_core[c]["idx"]
        m["dstl"] = per_core[c]["dstl"]
        in_maps.append(m)
    return in_maps


def assemble(results, cfg):
    out = np.empty((cfg.n_nodes, cfg.d_out), dtype=np.float32)
    for c in range(cfg.n_cores):
        out[c * cfg.npc:(c + 1) * cfg.npc, :] = results[c]["outT"].T[:cfg.npc]
    return out


def full_cfg():
    return Cfg(n_nodes=100000, n_cores=8, d_in=128, d_hid=128, d_out=64)


_CACHE = {}


def _install_ntff_hook():
    """Register the axon NTFF profiling hook if the image's antenv lacks it."""
    try:
        import types

        import antenv
        try:
            from antenv.axon_hooks import get_axon_ntff_profile_hook  # noqa: F401
            return
        except ImportError:
            pass
        from trn_agent_boot.trn_boot import _ntff_profile_via_ctypes
        mod = types.ModuleType("antenv.axon_hooks")
        state = {"hook": None}
        mod.set_axon_ntff_profile_hook = lambda h: state.__setitem__("hook", h)
        mod.get_axon_ntff_profile_hook = lambda: state["hook"]
        sys.modules["antenv.axon_hooks"] = mod
        antenv.axon_hooks = mod
        mod.set_axon_ntff_profile_hook(
            _ntff_profile_via_ctypes("/opt/axon/libaxon_pjrt.so"))
    except Exception as e:  # degrade to no tracing
        print(f"ntff hook install failed: {e}")


def kernel(x, edge_index, W1, b1, W2, b2, W3, b3):
    from concourse.bass_utils import run_bass_kernel_spmd

    cfg = full_cfg()
    sched, per_core = _host_prep(np.asarray(edge_index), cfg)
    key = "full"
    if key not in _CACHE:
        _CACHE[key] = build_nc(cfg, sched)
    nc = _CACHE[key]
    in_maps = make_in_maps(x, W1, b1, W2, b2, W3, b3, cfg, sched, per_core)
    trace = bool(int(os.environ.get("GCN_TRACE", "0")))
    if trace:
        _install_ntff_hook()
    res = run_bass_kernel_spmd(nc, in_maps, core_ids=list(range(cfg.n_cores)),
                               trace=trace)
    if res.exec_time_ns is not None:
        print(f"HW exec time: {res.exec_time_ns} ns")
    return assemble(res.results, cfg)


# revision 31
# speedup vs baseline: 1.0168x; 1.0168x over previous
"""3-layer GCN encoder (nn_GCNEncoder) on 8 Trainium2 NeuronCores.

Strategy (graph/data parallel, 1D node sharding):
  - Node shard c = rows [c*NPC, (c+1)*NPC).  Core c owns all edges whose
    *destination* lies in its shard (plus that shard's self-loops).
  - GCN norm is factorized:  out = dinv ⊙ (A^T (dinv ⊙ (h W))) + b, so no
    per-edge scaling is needed: dinv is applied once per node before the
    AllGather (on the transform output) and once per node at PSUM
    evacuation.
  - Per layer on each core:
      1. transform:  u = dinv ⊙ (h @ W), stored bf16 (layer 3's 64-wide
         rows are padded to 128 columns so gather rows stay 256B)
      2. AllGather u  ->  u_full [N, F] in local HBM (the gather table)
      3. aggregation: edges sorted by (superblock, src-group, dst-block,
         dst) and PACKED per (superblock, group) run: chunks of 128 edge
         slots may span dst-block boundaries.  dma_gather pulls the source
         rows (src-group slicing keeps gather indices within int16); each
         128-slot chunk is scattered into per-dst-block PSUM accumulators
         via one matmul per (chunk, block) "part":  psum[F, dst] +=
         msg^T @ S_part, where S_part is a 0/1 selection matrix built on
         DVE (is_equal against an iota constant) whose rows are zero for
         slots not belonging to that block.  Packing the run (instead of
         padding each (block, group) bucket to a chunk multiple) pools the
         max-over-cores variance and cuts gathered slots by ~13%.
      4. evacuation: h_next = relu(dinv ⊙ psum + b)
  - The SPMD instruction stream is shared by all 8 cores: run slot counts
    are the max over cores; pad slots gather row 0 with a -1 label (zero
    row in every S), contributing nothing.

kernel() takes the full unsharded inputs and returns the full output.
"""

import os
import sys

import numpy as np

sys.path.insert(0, "/opt/trn_rl_repo")

P = 128
GMAX = 1024          # max idxs per dma_gather (>=2048 wedges the NC)


class Cfg:
    def __init__(self, n_nodes, n_cores, d_in, d_hid, d_out,
                 sb_blocks=6, n_groups=4):
        assert n_nodes % n_cores == 0
        self.n_nodes = n_nodes
        self.n_cores = n_cores
        self.d_in, self.d_hid, self.d_out = d_in, d_hid, d_out
        self.npc = n_nodes // n_cores              # nodes per core
        self.nblk = -(-self.npc // P)              # dst blocks per core
        self.npcp = self.nblk * P                  # padded nodes per core
        self.sb_blocks = sb_blocks                 # dst blocks per superblock
        self.nsb = -(-self.nblk // sb_blocks)
        self.n_groups = n_groups                   # src-range groups
        assert n_cores % n_groups == 0
        self.cpg = n_cores // n_groups             # cores per src group
        self.grp = self.npcp * self.cpg            # padded rows per src group
        assert self.grp <= 32767, "src group must fit int16 gather indices"


def _host_prep(edge_index, cfg):
    """Shard edges, build the shared packed-run schedule and per-core data.

    Returns (sched, per_core).  sched (shared across cores):
      - runs: list per (sb, g) of dict(slots, gathers=[(n_idxs, n_chunks)],
              parts=[(chunk_in_run, block, stop)], icol0, pcol0)
      - tot16: total idx plane columns;  totparts: total part count
      - dinv
    per_core[c]: idx plane [128, tot16] int16, dstl plane [128, totparts] f32
    """
    n, ncores, npc = cfg.n_nodes, cfg.n_cores, cfg.npc
    ei = np.asarray(edge_index)
    src = ei[0]
    dst = ei[1]
    deg = (np.bincount(dst, minlength=n) + 1).astype(np.float64)
    dinv = (1.0 / np.sqrt(deg)).astype(np.float32)

    core = dst // npc
    # per-core sorted edge arrays and per-(sb,g,b) counts
    per_core_raw = []
    counts = np.zeros((ncores, cfg.nblk, cfg.n_groups), dtype=np.int64)
    for c in range(ncores):
        m = core == c
        s = src[m]
        d = (dst[m] - c * npc).astype(np.int64)
        blk = d // P
        sc = s // npc
        grp = sc // cfg.cpg
        sb = blk // cfg.sb_blocks
        order = np.lexsort((d, blk, grp, sb))
        s, d, blk, grp = s[order], d[order], blk[order], grp[order]
        sc = s // npc
        loc = (sc % cfg.cpg) * cfg.npcp + (s % npc)
        np.add.at(counts[c], (blk, grp), 1)
        per_core_raw.append((loc, d, blk, grp))

    # shared run schedule: per (sb, g) slots = 128*ceil(max_c count/128)
    runs = []
    tot16 = 0
    totparts = 0
    last_part_of_block = {}   # (block) -> part index (global)
    all_parts = []            # global part list in stream order
    for sb in range(cfg.nsb):
        blocks = list(range(sb * cfg.sb_blocks,
                            min((sb + 1) * cfg.sb_blocks, cfg.nblk)))
        for g in range(cfg.n_groups):
            cnt_cb = counts[:, blocks, g]            # [ncores, nb]
            run_max = int(cnt_cb.sum(axis=1).max())
            nch = max(1, -(-run_max // P))
            slots = nch * P
            # union over cores of (chunk, block) parts
            parts_set = set()
            for c in range(ncores):
                cum = 0
                for bi, b in enumerate(blocks):
                    cnt = int(counts[c, b, g])
                    if cnt == 0:
                        cum += 0
                        continue
                    lo, hi = cum, cum + cnt
                    for k in range(lo // P, -(-hi // P)):
                        parts_set.add((k, b))
                    cum = hi
            parts = sorted(parts_set)
            part_list = []
            for (k, b) in parts:
                gidx = len(all_parts)
                part_list.append([k, b, False])
                all_parts.append([sb, g, k, b])
                last_part_of_block[b] = gidx
            # gathers: split slots into <=GMAX idx pieces (multiples of 128)
            gathers = []
            rem = slots
            while rem > 0:
                take = min(GMAX, rem)
                gathers.append((take, take // P))
                rem -= take
            runs.append({
                "sb": sb, "g": g, "blocks": blocks, "slots": slots,
                "nch": nch, "gathers": gathers,
                "parts": part_list, "icol0": tot16, "pcol0": totparts,
            })
            tot16 += slots // 16
            totparts += len(part_list)
    # stop flags: statically-last part of each block
    for b, gidx in last_part_of_block.items():
        sb, g, k, bb = all_parts[gidx]
        for run in runs:
            if run["sb"] == sb and run["g"] == g:
                for pl in run["parts"]:
                    if pl[0] == k and pl[1] == b:
                        pl[2] = True

    # per-core planes
    per_core = []
    for c in range(ncores):
        loc, d, blk, grp = per_core_raw[c]
        # order index into sorted arrays by (sb, g): recompute segment ptrs
        sbv = blk // cfg.sb_blocks
        key = sbv * cfg.n_groups + grp
        # edges already sorted by (sb, g, blk, d) => key is nondecreasing
        idx_plane = np.zeros((16, tot16), dtype=np.int16)
        dstl_plane = np.full((P, totparts), -1.0, dtype=np.float32)
        ptr = 0
        for run in runs:
            sb, g = run["sb"], run["g"]
            kk = sb * cfg.n_groups + g
            lo = ptr + np.searchsorted(key[ptr:], kk, side="left")
            hi = ptr + np.searchsorted(key[ptr:], kk, side="right")
            ptr = hi
            cnt = hi - lo
            slots = run["slots"]
            assert cnt <= slots
            arr = np.zeros(slots, dtype=np.int16)
            arr[:cnt] = loc[lo:hi].astype(np.int16)
            a16 = arr.reshape(slots // 16, 16).T      # [16, slots/16]
            idx_plane[:, run["icol0"]:run["icol0"] + slots // 16] = a16
            # labels per part
            dl = d[lo:hi]
            bl = blk[lo:hi]
            for j, (k, b, _stop) in enumerate(run["parts"]):
                w0, w1 = k * P, (k + 1) * P
                col = np.full(P, -1.0, dtype=np.float32)
                lo2, hi2 = max(w0, 0), min(w1, cnt)
                if lo2 < hi2:
                    seg = slice(lo2, hi2)
                    mask = bl[seg] == b
                    rel = np.nonzero(mask)[0]
                    if rel.size:
                        col[(lo2 - w0) + rel] = (dl[seg][mask] - b * P)
                dstl_plane[:, run["pcol0"] + j] = col
        assert ptr == len(loc)
        idx_wrapped = np.ascontiguousarray(np.tile(idx_plane, (8, 1)))
        per_core.append({"idx": idx_wrapped,
                         "dstl": np.ascontiguousarray(dstl_plane)})

    maxparts_g = 0
    for run in runs:
        # parts per gather window (for st batch sizing)
        ch0 = 0
        for (n_idx, n_ch) in run["gathers"]:
            pc = sum(1 for (k, b, _s) in run["parts"]
                     if ch0 <= k < ch0 + n_ch)
            maxparts_g = max(maxparts_g, pc)
            ch0 += n_ch
    sched = {
        "runs": runs, "tot16": tot16, "totparts": totparts,
        "maxrun16": max(r["slots"] // 16 for r in runs),
        "maxparts_g": maxparts_g,
        "dinv": dinv,
        "nch_by_block": counts.max(axis=0),   # [nblk, n_groups] edge counts
    }
    return sched, per_core


def build_nc(cfg, sched, debug=False):
    from concourse import bacc, mybir

    f32 = mybir.dt.float32
    bf16 = mybir.dt.bfloat16
    i16 = mybir.dt.int16
    Alu = mybir.AluOpType
    Act = mybir.ActivationFunctionType

    npc, nblk = cfg.npc, cfg.nblk
    tot16, totparts = sched["tot16"], sched["totparts"]
    runs = sched["runs"]
    maxrun16 = sched["maxrun16"]
    maxparts_g = sched["maxparts_g"]
    layer_dims = [(cfg.d_in, cfg.d_hid), (cfg.d_hid, cfg.d_hid),
                  (cfg.d_hid, cfg.d_out)]
    # u is bf16 for all layers; layer 3 (fo=64) pads rows to UC=128 columns
    # so gather rows stay 256B (the 64 garbage columns are never read)
    u_dt = [bf16, bf16, bf16]
    UC = P

    nc = bacc.Bacc("TRN2", target_bir_lowering=False, debug=debug,
                   enable_asserts=False, num_devices=cfg.n_cores)

    xT = nc.dram_tensor("xT", [P, cfg.npcp], f32, kind="ExternalInput")
    Wd, Bd = [], []
    for li, (fi, fo) in enumerate(layer_dims):
        Wd.append(nc.dram_tensor(f"W{li + 1}", [fi, fo], f32, kind="ExternalInput"))
        Bd.append(nc.dram_tensor(f"B{li + 1}", [fo, 1], f32, kind="ExternalInput"))
    dinv_col_d = nc.dram_tensor("dinv_col", [P, nblk], f32, kind="ExternalInput")
    dinvb_d = nc.dram_tensor("dinvb", [P, cfg.npcp], f32, kind="ExternalInput")
    iota_d = nc.dram_tensor("iota_t", [P, maxparts_g * P], f32,
                            kind="ExternalInput")
    ident_d = nc.dram_tensor("ident", [P, P], f32, kind="ExternalInput")
    idx_d = nc.dram_tensor("idxs", [P, tot16], i16, kind="ExternalInput")
    dstl_d = nc.dram_tensor("dstl", [P, totparts], f32, kind="ExternalInput")
    outT = nc.dram_tensor("outT", [cfg.d_out, cfg.npcp], f32,
                          kind="ExternalOutput")

    u_own, u_full = [], []
    for li, (fi, fo) in enumerate(layer_dims):
        u_own.append(nc.dram_tensor(f"u_own{li + 1}", [cfg.npcp, UC],
                                    u_dt[li]))
        u_full.append(nc.dram_tensor(f"u_full{li + 1}",
                                     [cfg.n_cores * cfg.npcp, UC], u_dt[li],
                                     addr_space="Shared"))

    from concourse import tile

    rg = [list(range(cfg.n_cores))]
    with tile.TileContext(nc) as tc:
        with (
            tc.tile_pool(name="const", bufs=1) as constp,
            tc.tile_pool(name="hbuf", bufs=1) as hp,
            tc.tile_pool(name="gath", bufs=4) as gp,
            tc.tile_pool(name="gidx", bufs=3) as ip,
            tc.tile_pool(name="sel", bufs=4) as sp,
            tc.tile_pool(name="dinvb", bufs=2) as dbp,
            tc.tile_pool(name="evac", bufs=3) as tp,
            tc.tile_pool(name="ustage", bufs=3) as up,
            tc.tile_pool(name="accp", bufs=cfg.sb_blocks, space="PSUM") as accp,
            tc.tile_pool(name="auxp", bufs=2, space="PSUM") as auxp,
        ):
            from concourse import library_config
            nc.gpsimd.load_library(library_config.mlp)

            # constants
            wt, bt = [], []
            for li, (fi, fo) in enumerate(layer_dims):
                w = constp.tile([fi, fo], f32, tag=f"w{li}")
                nc.sync.dma_start(w[:], Wd[li][:])
                wt.append(w)
                b = constp.tile([fo, 1], f32, tag=f"b{li}")
                nc.sync.dma_start(b[:], Bd[li][:])
                bt.append(b)
            dct = constp.tile([P, nblk], f32, tag="dct")
            nc.sync.dma_start(dct[:], dinv_col_d[:])
            iot = constp.tile([P, maxparts_g * P], f32, tag="iot")
            nc.sync.dma_start(iot[:], iota_d[:])
            idt = constp.tile([P, P], f32, tag="idt")
            nc.sync.dma_start(idt[:], ident_d[:])
            dlt = constp.tile([P, totparts], f32, tag="dlt")
            nc.sync.dma_start(dlt[:], dstl_d[:])
            # bf16 copies for the bf16 aggregation path (cast on device)
            iot_b = constp.tile([P, maxparts_g * P], bf16, tag="iot_b")
            nc.vector.tensor_scalar_add(iot_b[:], iot[:], 0.0)
            idt_b = constp.tile([P, P], bf16, tag="idt_b")
            nc.vector.tensor_scalar_add(idt_b[:], idt[:], 0.0)
            dlt_b = constp.tile([P, totparts], bf16, tag="dlt_b")
            nc.vector.tensor_scalar_add(dlt_b[:], dlt[:], 0.0)

            h = hp.tile([P, cfg.npcp], f32, tag="h")
            nc.sync.dma_start(h[:], xT[:])

            def emit_transform_block(li, b):
                fi, fo = layer_dims[li]
                off = b * P
                pt = auxp.tile([P, P], f32, tag="aux")
                nc.tensor.matmul(pt[:P, :fo], lhsT=h[:fi, off:off + P],
                                 rhs=wt[li][:, :fo], start=True, stop=True)
                ut = up.tile([P, P], u_dt[li], tag=f"u{li}")
                nc.vector.tensor_scalar_mul(ut[:P, :fo], pt[:P, :fo],
                                            dct[:P, b:b + 1])
                nc.sync.dma_start(u_own[li][off:off + P, :fo], ut[:P, :fo])

            # layer 1 transform (later layers' transforms are interleaved
            # into the previous layer's evacuation loop)
            for b in range(nblk):
                emit_transform_block(0, b)

            for li, (fi, fo) in enumerate(layer_dims):
                last_layer = li == len(layer_dims) - 1
                ud = u_dt[li]
                idt_l = idt if ud == f32 else idt_b
                iot_l = iot if ud == f32 else iot_b
                dlt_l = dlt if ud == f32 else dlt_b

                # ---- AllGather the transformed features ----
                nc.gpsimd.collective_compute(
                    "AllGather", mybir.AluOpType.bypass, replica_groups=rg,
                    ins=[u_own[li][:]], outs=[u_full[li][:]],
                )

                # ---- aggregation (packed runs) ----
                for sb in range(cfg.nsb):
                    blocks = list(range(sb * cfg.sb_blocks,
                                        min((sb + 1) * cfg.sb_blocks, nblk)))
                    # self-loop contribution opens each block's accumulation
                    sb_off = blocks[0] * P
                    nfull = len(blocks)
                    ublk = gp.tile([P, cfg.sb_blocks * P], ud, tag=f"ublk{li}",
                                   name=f"ublk{li}_{sb}")
                    nc.sync.dma_start(
                        ublk[:, :nfull * fo].rearrange(
                            "p (c f) -> p c f", f=fo),
                        u_own[li][sb_off:sb_off + nfull * P, :fo].rearrange(
                            "(c p) f -> p c f", p=P))
                    acc = {}
                    has_parts = {b: False for b in blocks}
                    for run in runs:
                        if run["sb"] == sb:
                            for (k, b, _s) in run["parts"]:
                                has_parts[b] = True
                    for b in blocks:
                        ci = b - blocks[0]
                        acc[b] = accp.tile([P, P], f32, tag="acc",
                                           name=f"acc{li}_{b}")
                        nc.tensor.matmul(
                            acc[b][:fo, :],
                            lhsT=ublk[:, ci * fo:ci * fo + fo],
                            rhs=idt_l[:, :],
                            start=True,
                            stop=not has_parts[b],
                        )
                    for g in range(cfg.n_groups):
                        run = runs[sb * cfg.n_groups + g]
                        slots = run["slots"]
                        l16 = slots // 16
                        it = ip.tile([P, maxrun16], i16, tag="it")
                        nc.sync.dma_start(
                            it[:, :l16],
                            idx_d[:, run["icol0"]:run["icol0"] + l16])
                        ch0 = 0
                        for (n_idx, n_ch) in run["gathers"]:
                            gt = gp.tile([P, (GMAX // P) * UC], ud,
                                         tag=f"gt{li}")
                            nc.gpsimd.dma_gather(
                                out_ap=gt[:, :n_ch * UC].rearrange(
                                    "p (c e) -> p c e", e=UC),
                                in_ap=u_full[li][g * cfg.grp:(g + 1) * cfg.grp, :],
                                idxs_ap=it[:, ch0 * 8:(ch0 + n_ch) * 8],
                                num_idxs=n_idx,
                                num_idxs_reg=n_idx,
                                elem_size=UC,
                                single_packet=False,
                            )
                            gparts = [(j, k, b, stop) for j, (k, b, stop)
                                      in enumerate(run["parts"])
                                      if ch0 <= k < ch0 + n_ch]
                            if gparts:
                                npg = len(gparts)
                                st = sp.tile([P, maxparts_g * P], ud,
                                             tag=f"st{li}")
                                # batched one-hot build: one is_equal over
                                # all parts of this gather
                                pc0 = run["pcol0"] + gparts[0][0]
                                in1 = dlt_l[:, pc0:pc0 + npg].rearrange(
                                    "p (c o) -> p c o", o=1).to_broadcast(
                                        [P, npg, P])
                                nc.vector.tensor_tensor(
                                    out=st[:, :npg * P].rearrange(
                                        "p (c e) -> p c e", e=P),
                                    in0=iot_l[:, :npg * P].rearrange(
                                        "p (c e) -> p c e", e=P),
                                    in1=in1,
                                    op=Alu.is_equal,
                                )
                                for jj, (j, k, b, stop) in enumerate(gparts):
                                    nc.tensor.matmul(
                                        acc[b][:fo, :],
                                        lhsT=gt[:, (k - ch0) * UC:
                                                (k - ch0) * UC + fo],
                                        rhs=st[:, jj * P:(jj + 1) * P],
                                        start=False, stop=stop,
                                    )
                            ch0 += n_ch

                    # ---- evacuate superblock ----
                    sb_off = blocks[0] * P
                    sb_w = (blocks[-1] + 1) * P - sb_off
                    dbt = dbp.tile([P, cfg.sb_blocks * P], f32, tag="dbt")
                    nc.sync.dma_start(dbt[:, :sb_w],
                                      dinvb_d[:, sb_off:sb_off + sb_w])
                    for b in blocks:
                        off = b * P
                        tt = tp.tile([P, P], f32, tag="tt")
                        nc.vector.tensor_tensor(
                            tt[:fo, :P], in0=acc[b][:fo, :P],
                            in1=dbt[:fo, off - sb_off:off - sb_off + P],
                            op=Alu.mult)
                        if not last_layer:
                            nc.scalar.activation(h[:fo, off:off + P],
                                                 tt[:fo, :P], Act.Relu,
                                                 bias=bt[li][:, :1])
                            # next layer's transform for this block, fed by
                            # the h columns just written
                            emit_transform_block(li + 1, b)
                        else:
                            ot = up.tile([P, P], f32, tag="uo")
                            nc.vector.tensor_scalar_add(ot[:fo, :P],
                                                        tt[:fo, :P],
                                                        bt[li][:, :1])
                            nc.sync.dma_start(outT[:, off:off + P],
                                              ot[:fo, :P])

    nc.finalize()
    return nc


def make_in_maps(x, W1, b1, W2, b2, W3, b3, cfg, sched, per_core):
    x = np.ascontiguousarray(np.asarray(x, dtype=np.float32))
    dinv = sched["dinv"]
    npc, nblk = cfg.npc, cfg.nblk
    iota = np.tile(np.arange(P, dtype=np.float32),
                   (P, sched["maxparts_g"]))
    common = {
        "W1": np.ascontiguousarray(np.asarray(W1, np.float32)),
        "W2": np.ascontiguousarray(np.asarray(W2, np.float32)),
        "W3": np.ascontiguousarray(np.asarray(W3, np.float32)),
        "B1": np.asarray(b1, np.float32).reshape(-1, 1).copy(),
        "B2": np.asarray(b2, np.float32).reshape(-1, 1).copy(),
        "B3": np.asarray(b3, np.float32).reshape(-1, 1).copy(),
        "iota_t": np.ascontiguousarray(iota),
        "ident": np.eye(P, dtype=np.float32),
    }
    in_maps = []
    for c in range(cfg.n_cores):
        dv_pad = np.zeros(cfg.npcp, np.float32)
        dv_pad[:npc] = dinv[c * npc:(c + 1) * npc]
        xT = np.zeros((P, cfg.npcp), np.float32)
        xT[:, :npc] = x[c * npc:(c + 1) * npc].T
        m = dict(common)
        m["xT"] = xT
        m["dinv_col"] = np.ascontiguousarray(dv_pad.reshape(nblk, P).T)
        m["dinvb"] = np.ascontiguousarray(np.broadcast_to(dv_pad, (P, cfg.npcp)))
        m["idxs"] = per_core[c]["idx"]
        m["dstl"] = per_core[c]["dstl"]
        in_maps.append(m)
    return in_maps


def assemble(results, cfg):
    out = np.empty((cfg.n_nodes, cfg.d_out), dtype=np.float32)
    for c in range(cfg.n_cores):
        out[c * cfg.npc:(c + 1) * cfg.npc, :] = results[c]["outT"].T[:cfg.npc]
    return out


def full_cfg():
    return Cfg(n_nodes=100000, n_cores=8, d_in=128, d_hid=128, d_out=64)


_CACHE = {}


def _install_ntff_hook():
    """Register the axon NTFF profiling hook if the image's antenv lacks it."""
    try:
        import types

        import antenv
        try:
            from antenv.axon_hooks import get_axon_ntff_profile_hook  # noqa: F401
            return
        except ImportError:
            pass
        from trn_agent_boot.trn_boot import _ntff_profile_via_ctypes
        mod = types.ModuleType("antenv.axon_hooks")
        state = {"hook": None}
        mod.set_axon_ntff_profile_hook = lambda h: state.__setitem__("hook", h)
        mod.get_axon_ntff_profile_hook = lambda: state["hook"]
        sys.modules["antenv.axon_hooks"] = mod
        antenv.axon_hooks = mod
        mod.set_axon_ntff_profile_hook(
            _ntff_profile_via_ctypes("/opt/axon/libaxon_pjrt.so"))
    except Exception as e:  # degrade to no tracing
        print(f"ntff hook install failed: {e}")


def kernel(x, edge_index, W1, b1, W2, b2, W3, b3):
    from concourse.bass_utils import run_bass_kernel_spmd

    cfg = full_cfg()
    sched, per_core = _host_prep(np.asarray(edge_index), cfg)
    key = "full"
    if key not in _CACHE:
        _CACHE[key] = build_nc(cfg, sched)
    nc = _CACHE[key]
    in_maps = make_in_maps(x, W1, b1, W2, b2, W3, b3, cfg, sched, per_core)
    trace = bool(int(os.environ.get("GCN_TRACE", "0")))
    if trace:
        _install_ntff_hook()
    res = run_bass_kernel_spmd(nc, in_maps, core_ids=list(range(cfg.n_cores)),
                               trace=trace)
    if res.exec_time_ns is not None:
        print(f"HW exec time: {res.exec_time_ns} ns")
    return assemble(res.results, cfg)
